# revision 25
# baseline (speedup 1.0000x reference)
"""Batch-hard triplet loss on 8 Trainium2 NeuronCores.

Math (matches the reference up to fp rounding and a tiny truncation noise):
  d_ij   = ||h_i||^2 + ||h_j||^2 - 2 h_i.h_j, clamped to [EPS, inf)
  hp_i   = max over j (same label, j != i) of d_ij
  hn_i   = 2nd-smallest over j (different label) of d_ij
  loss_i = max(hp_i - hn_i + ALPHA, 0)
  out    = sum(loss_i[loss_i > EPS]) / count(loss_i > EPS)

Device strategy: rows are sharded over 8 cores (1024 each). Each core mines
from the quantity

  p_ij = 2 h_i.h_j - ||h_j||^2 - BIG * [label_i == label_j]

Row-constant terms (||h_i||^2, the EPS clamp) cancel in hp - hn, so they are
never computed.  With t_ij := d_ij - ||h_i||^2 = -p_ij - BIG*eq:
  hp_i = -min_j(p_ij) - BIG        (positives carry -BIG, dominate the min;
                                    Sterbenz: the BIG subtraction is exact)
  hn_i = -max8(p_i)[1]             (negatives are the largest p; the DVE Max8
                                    instruction gives the top-8 descending, so
                                    element 1 is the 2nd-smallest distance,
                                    with tie multiplicity matching top_k)
  loss_i = max( max8[1] - min + (ALPHA - BIG), 0 )   (clamp applied on host)

The key trick: the whole p_ij for an unmasked block comes out of FOUR e4m3
DoubleRow matmuls (K=256 each).  The last 3 of the 1024 contraction slots
carry not data but a 3-term e4m3 expansion of ||h_j||^2 against a constant
-4 on the A side (4*e4m3(x/4) greedy residuals leave |err| <= 1/16), so the
per-column norm rides the GEMM for free.  The 3 sacrificed data dims add
zero-mean noise (std ~3.5) to each distance, far below the mining gaps.

Rows are sorted by label and each core's B columns are rotated so every
own-class (positive) column sits in the first hpj = 3 j-blocks.  Only those
blocks append a 5th e5m2 DoubleRow matmul with the -BIG one-hot mask.

The j loop walks PAIRS of 512-column blocks per row chunk so each PSUM tile
spans two banks ([128, 1024] f32); one DVE Max8 covers the pair, halving the
DVE instruction count.  Hardest-positive minima read the masked halves
directly.  Per-row losses leave the device as a [128, m_chunks] tile
(partition p, row chunk m) in one contiguous DMA; the host transposes and
does the masked mean.
"""

import functools

import numpy as np
import ml_dtypes

import concourse.bacc as bacc
import concourse.tile as tile
from concourse import mybir
from concourse.bass_utils import run_bass_kernel_spmd

FP8E4 = mybir.dt.float8e4
FP8E5 = mybir.dt.float8e5
F32 = mybir.dt.float32
BF16 = mybir.dt.bfloat16
E4 = ml_dtypes.float8_e4m3
E5 = ml_dtypes.float8_e5m2

N, D, C = 8192, 1024, 128
NCORES = 8
P = 128
JB = 512  # matmul moving free dim = one fp32 PSUM bank
HPJ = 3   # j-blocks that can contain positive (own-class) columns
ALPHA = 0.1
EPS = 1e-7
BIG = 8192.0
NNORM = 3   # e4m3 norm-expansion slots stolen from the contraction
NSCALE = 8.0  # A-side constant: each slot contributes -8 * e4m3(r/8)
              # (this e4m3 flavor saturates at 240, so r0/8 <= ~165 fits)
MODE = "fp8"


def build_program(rows, n, d, c, jb, mode=MODE, psum_bufs=4, b_bufs=5):
    """Emit the per-core Bass/Tile program (identical on all cores)."""
    kh = d // P
    m_chunks = rows // P
    nj = n // jb
    hpj = min(nj, HPJ)
    assert rows % P == 0 and d % P == 0 and n % jb == 0 and c <= P
    assert kh % 2 == 0 and nj % 2 == 0

    nc = bacc.Bacc("TRN2", target_bir_lowering=False)
    A4 = nc.dram_tensor("A4", [P, kh, rows], FP8E4, kind="ExternalInput")
    B4 = nc.dram_tensor("B4", [P, kh, n], FP8E4, kind="ExternalInput")
    A5 = nc.dram_tensor("A5", [P, 2, rows], FP8E5, kind="ExternalInput")
    B5 = nc.dram_tensor("B5", [P, 2, hpj * jb], FP8E5, kind="ExternalInput")
    loss = nc.dram_tensor("loss", [P, m_chunks], F32, kind="ExternalOutput")

    with tile.TileContext(nc) as tc:
        with (
            tc.tile_pool(name="apool", bufs=1) as apool,
            tc.tile_pool(name="bpool", bufs=b_bufs) as bpool,
            tc.tile_pool(name="psum", bufs=psum_bufs, space="PSUM") as pp,
            tc.tile_pool(name="mpool", bufs=1) as mpool,
            tc.tile_pool(name="fpool", bufs=6) as fpool,
        ):
            # Warm the PE HAM clock gate while the first DMAs land: dummy
            # matmuls on a zeroed tile keep the PE busy through its
            # 4096-cycle activity window so real matmuls run at 2.4 GHz.
            wsrc = apool.tile([1, 16 + jb], BF16, tag="wsrc")
            nc.vector.memset(wsrc[:], 0.0)
            wps = pp.tile([P, 2 * jb], F32, name="ps", tag="ps")
            for _ in range(5):
                nc.tensor.matmul(wps[:16, :jb], wsrc[:1, :16], wsrc[:1, 16:],
                                 start=True, stop=True)

            kq = kh // 2  # DoubleRow matmuls (k-tile pairs) per data block

            # DMA triggers cost ~0.8us of issuing-engine time each, so the
            # head-critical loads are spread over three otherwise-idle
            # queues: Sync streams B, Scalar streams the stationary A
            # chunks, GpSimd takes the small mask operands.
            def load_bpair(p):
                """Load j-blocks (2p, 2p+1) as one DMA; [[APs], [APs]]."""
                js = slice(2 * p * jb, (2 * p + 2) * jb)
                b4 = bpool.tile([P, kh, 2 * jb], FP8E4, tag="b4", name="b4")
                nc.sync.dma_start(out=b4[:], in_=B4[:, :, js])
                return [[b4[:, 2 * t:2 * t + 2, h * jb:(h + 1) * jb]
                         for t in range(kq)] for h in range(2)]

            def load_b1(j):
                """Load a single j-block as one DMA (head blocks)."""
                js = slice(j * jb, (j + 1) * jb)
                b4 = bpool.tile([P, kh, jb], FP8E4, tag="b4s", name="b4s")
                nc.sync.dma_start(out=b4[:], in_=B4[:, :, js])
                return [b4[:, 2 * t:2 * t + 2, :] for t in range(kq)]

            # Process an UNMASKED pair first: its tiles need no mask
            # operands, so the small A5/B5 transfers get ~20us of slack on
            # the (slow-to-start) GpSimd queue, and the masked pairs run in
            # windows 1-2 -- early enough that their hp minima stay clear
            # of the final-merge window.
            order = [2, 0, 1] + list(range(3, nj // 2))

            # Head loads, one DMA per tensor, interleaved on the Sync queue
            # in consumption order.  Trigger issue (~0.75us apiece on Sync;
            # ~3.5us on Scalar, so everything head-critical stays on Sync)
            # dominates the head, so fewer/bigger DMAs beat split ones.
            bpair = [None] * (nj // 2)
            js0 = slice(2 * order[0] * jb, (2 * order[0] + 1) * jb)
            b0lo = bpool.tile([P, kq, jb], FP8E4, tag="b4s", name="b4s")
            nc.sync.dma_start(out=b0lo[:], in_=B4[:, :kq, js0])
            a0lo = apool.tile([P, kq, P], FP8E4, tag="a4m0lo", name="a4m0lo")
            nc.sync.dma_start(out=a0lo[:], in_=A4[:, :kq, 0:P])
            b5all = apool.tile([P, 2, hpj * jb], FP8E5, tag="b5all")
            nc.gpsimd.dma_start(out=b5all[:], in_=B5[:])
            b0hi = bpool.tile([P, kq, jb], FP8E4, tag="b4s", name="b4s")
            nc.sync.dma_start(out=b0hi[:], in_=B4[:, kq:, js0])
            a0hi = apool.tile([P, kq, P], FP8E4, tag="a4m0hi", name="a4m0hi")
            nc.sync.dma_start(out=a0hi[:], in_=A4[:, kq:, 0:P])
            a5 = apool.tile([P, 2, rows], FP8E5, tag="a5", name="a5")
            nc.gpsimd.dma_start(out=a5[:], in_=A5[:])
            b0 = [b0lo[:, 0:2, :], b0lo[:, 2:4, :],
                  b0hi[:, 0:2, :], b0hi[:, 2:4, :]]
            b1 = load_b1(2 * order[0] + 1)
            a4aps = [[a0lo[:, 0:2, :], a0lo[:, 2:4, :],
                      a0hi[:, 0:2, :], a0hi[:, 2:4, :]]]
            for m in range(1, m_chunks):
                ms = slice(m * P, (m + 1) * P)
                t = apool.tile([P, kh, P], FP8E4, tag=f"a4m{m}",
                               name=f"a4m{m}")
                nc.sync.dma_start(out=t[:], in_=A4[:, :, ms])
                a4aps.append([t[:, 2 * k:2 * k + 2, :] for k in range(kq)])
            bpair[order[0]] = [b0, b1]

            def b5ap(j):
                return b5all[:, :, j * jb:(j + 1) * jb]

            # Per-row-chunk partial mining results, merged after the j loop.
            # Max8 runs once per PSUM pair of j-blocks.
            v8 = [mpool.tile([P, nj * 4], F32, tag=f"v8_{m}", name=f"v8_{m}")
                  for m in range(m_chunks)]
            gmin = [mpool.tile([P, hpj], F32, tag=f"gm_{m}", name=f"gmin_{m}")
                    for m in range(m_chunks)]

            stage8 = mpool.tile([P, m_chunks], F32, tag="stage8")
            # hp mining of the masked halves is deferred: the Act engine
            # drains each masked half to SBUF during its (DVE-saturated)
            # window, and the [P, jb] MIN reductions run in later windows
            # where the DVE is half idle.
            pend_min = []

            for w, jp in enumerate(order):
                if bpair[jp] is None:
                    bpair[jp] = load_bpair(jp)
                if w + 1 < len(order) and bpair[order[w + 1]] is None:
                    # Keep the moving stream one pair ahead of the PE.
                    bpair[order[w + 1]] = load_bpair(order[w + 1])
                last = w == len(order) - 1

                for m in range(m_chunks):
                    ps = pp.tile([P, 2 * jb], F32, name="ps", tag="ps")
                    at = a4aps[m]
                    for half in range(2):
                        j = 2 * jp + half
                        b4aps = bpair[jp][half]
                        ph = ps[:, half * jb:(half + 1) * jb]
                        for t in range(kq):
                            nc.tensor.matmul(
                                ph, at[t], b4aps[t],
                                start=(t == 0),
                                stop=(t == kq - 1 and j >= hpj),
                                perf_mode=mybir.MatmulPerfMode.DoubleRow)
                        if j < hpj:
                            # Masked block: -BIG one-hot rides a 5th matmul.
                            nc.tensor.matmul(
                                ph, a5[:, :, m * P:(m + 1) * P], b5ap(j),
                                start=False, stop=True,
                                perf_mode=mybir.MatmulPerfMode.DoubleRow)
                            sc = mpool.tile([P, jb], F32, tag=f"sc{m}_{j}",
                                            name=f"sc{m}_{j}")
                            nc.scalar.copy(sc[:], ph)
                            pend_min.append((sc, m, j))

                    nc.vector.max(v8[m][:, jp * 8:(jp + 1) * 8], ps[:])

                    if 3 <= w < 6 and pend_min:
                        sc, mm, mj = pend_min.pop(0)
                        nc.vector.tensor_reduce(gmin[mm][:, mj:mj + 1], sc[:],
                                                axis=mybir.AxisListType.X,
                                                op=mybir.AluOpType.min)

                    if last:
                        # Final merge for this row chunk, interleaved so it
                        # overlaps the remaining row chunks' matmuls.
                        vf = fpool.tile([P, 8], F32, tag="vf", name="vf")
                        nc.vector.max(vf[:], v8[m][:])
                        gm = fpool.tile([P, 1], F32, tag="gm", name="gm")
                        nc.vector.tensor_reduce(gm[:], gmin[m][:],
                                                axis=mybir.AxisListType.X,
                                                op=mybir.AluOpType.min)
                        # loss_pre = (v2 + (ALPHA - BIG)) - gmin
                        nc.vector.scalar_tensor_tensor(
                            out=stage8[:, m:m + 1], in0=vf[:, 1:2],
                            scalar=float(ALPHA - BIG), in1=gm[:],
                            op0=mybir.AluOpType.add,
                            op1=mybir.AluOpType.subtract)
                        if m == m_chunks // 2 - 1:
                            # First output half leaves while the rest of the
                            # row chunks are still merging.
                            nc.scalar.dma_start(
                                out=loss[:, :m_chunks // 2],
                                in_=stage8[:, :m_chunks // 2])

            assert not pend_min, "deferred hp minima must drain before merge"
            nc.sync.dma_start(out=loss[:, m_chunks // 2:],
                              in_=stage8[:, m_chunks // 2:])

    nc.compile()
    return nc


def _split_e4(x, terms, scale):
    """Greedy expansion: x ~ scale * sum of `terms` e4m3 rows (f64 in/out)."""
    out = []
    r = x.astype(np.float64).copy()
    for _ in range(terms):
        s = (r / scale).astype(E4)
        out.append(s)
        r -= scale * s.astype(np.float64)
    return out


def make_inputs(H, labels, n, d, c, ncores, mode=MODE):
    """Host-side packing of the augmented GEMM operands.

    Rows are sorted by label and sharded contiguously.  Each core's B
    columns are rotated so every column whose label appears among that
    core's rows sits in the leading block (always < HPJ * JB columns), which
    lets the device mine the hardest positive from the first HPJ j-blocks
    only and skip the mask matmul everywhere else.  The final masked mean
    is permutation invariant, so neither the sort nor the rotations need
    undoing.

    The last NNORM contraction slots of A4/B4 are repurposed: A-side holds
    the constant -NSCALE, B-side the greedy e4m3 expansion of ||h_j||^2
    (computed over the SURVIVING d - NNORM dims' quantized values plus the
    full-precision tail, see below), so p_ij needs no extra matmul.
    """
    H = np.ascontiguousarray(np.asarray(H, dtype=np.float32))
    labels = np.asarray(labels).astype(np.int64).ravel()
    kh = d // P
    rows = n // ncores
    nj = n // JB
    hpj = min(nj, HPJ)

    perm = np.argsort(labels, kind="stable")
    H = H[perm]
    labels = labels[perm]
    col_orders = []
    for cix in range(ncores):
        own = np.zeros(n, dtype=bool)
        own[np.isin(labels, labels[cix * rows:(cix + 1) * rows])] = True
        order = np.concatenate([np.nonzero(own)[0], np.nonzero(~own)[0]])
        assert own.sum() <= min(n, hpj * JB), own.sum()
        col_orders.append(order)

    oh = labels[None, :] == np.arange(c, dtype=np.int64)[:, None]  # [c, n]

    Hr = H.astype(E4)
    # Full-data norm (all d dims, at e4m3 precision) -- matches the
    # reference's ||h||^2 term; the dot product just loses the last NNORM
    # dims (zero-mean noise on each distance).
    xn = np.einsum("ij,ij->i", Hr.astype(np.float64), Hr.astype(np.float64))
    xsplit = _split_e4(xn, NNORM, NSCALE)

    B4m = Hr.T.reshape(kh, P, n).transpose(1, 0, 2).copy()  # [P, kh, n] e4m3
    for t in range(NNORM):
        B4m[P - NNORM + t, kh - 1, :] = xsplit[t]
    B5m = np.zeros((P, 2, n), dtype=E5)
    B5m[:c, 0, :] = oh.astype(E5)

    in_maps = []
    for cix in range(ncores):
        sl = slice(cix * rows, (cix + 1) * rows)
        order = col_orders[cix]
        A4m = ((2.0 * Hr.astype(np.float32)[sl].T).astype(E4)
               .reshape(kh, P, rows).transpose(1, 0, 2).copy())
        A4m[P - NNORM:, kh - 1, :] = -NSCALE
        A5m = np.zeros((P, 2, rows), dtype=E5)
        A5m[:c, 0, :] = (-BIG * oh[:, sl]).astype(E5)
        in_maps.append({"A4": A4m, "B4": B4m[:, :, order],
                        "A5": A5m, "B5": B5m[:, :, order[:hpj * JB]]})
    return in_maps


@functools.lru_cache(maxsize=2)
def _get_program(mode=MODE):
    return build_program(N // NCORES, N, D, C, JB, mode=mode)


def _finalize(loss_rows):
    loss_all = np.concatenate(
        [np.asarray(l, dtype=np.float64).T.ravel() for l in loss_rows])
    loss_all = np.maximum(loss_all, 0.0)
    rel = loss_all > EPS
    cnt = int(rel.sum())
    if cnt == 0:
        return np.float32(0.0)
    return np.float32(loss_all[rel].sum() / cnt)


def kernel(H, labels):
    in_maps = make_inputs(H, labels, N, D, C, NCORES)
    res = run_bass_kernel_spmd(_get_program(), in_maps, list(range(NCORES)))
    return _finalize([r["loss"] for r in res.results])


# revision 27
# speedup vs baseline: 1.0051x; 1.0051x over previous
"""Batch-hard triplet loss on 8 Trainium2 NeuronCores.

Math (matches the reference up to fp rounding and a tiny truncation noise):
  d_ij   = ||h_i||^2 + ||h_j||^2 - 2 h_i.h_j, clamped to [EPS, inf)
  hp_i   = max over j (same label, j != i) of d_ij
  hn_i   = 2nd-smallest over j (different label) of d_ij
  loss_i = max(hp_i - hn_i + ALPHA, 0)
  out    = sum(loss_i[loss_i > EPS]) / count(loss_i > EPS)

Device strategy: rows are sharded over 8 cores (1024 each). Each core mines
from the quantity

  p_ij = 2 h_i.h_j - ||h_j||^2 - BIG * [label_i == label_j]

Row-constant terms (||h_i||^2, the EPS clamp) cancel in hp - hn, so they are
never computed.  With t_ij := d_ij - ||h_i||^2 = -p_ij - BIG*eq:
  hp_i = -min_j(p_ij) - BIG        (positives carry -BIG, dominate the min;
                                    Sterbenz: the BIG subtraction is exact)
  hn_i = -max8(p_i)[1]             (negatives are the largest p; the DVE Max8
                                    instruction gives the top-8 descending, so
                                    element 1 is the 2nd-smallest distance,
                                    with tie multiplicity matching top_k)
  loss_i = max( max8[1] - min + (ALPHA - BIG), 0 )   (clamp applied on host)

The key trick: the whole p_ij for an unmasked block comes out of FOUR e4m3
DoubleRow matmuls (K=256 each).  The last 3 of the 1024 contraction slots
carry not data but a 3-term e4m3 expansion of ||h_j||^2 against a constant
-4 on the A side (4*e4m3(x/4) greedy residuals leave |err| <= 1/16), so the
per-column norm rides the GEMM for free.  The 3 sacrificed data dims add
zero-mean noise (std ~3.5) to each distance, far below the mining gaps.

Rows are sorted by label and each core's B columns are rotated so every
own-class (positive) column sits in the first hpj = 3 j-blocks.  Only those
blocks append a 5th e5m2 DoubleRow matmul with the -BIG one-hot mask.

The j loop walks PAIRS of 512-column blocks per row chunk so each PSUM tile
spans two banks ([128, 1024] f32); one DVE Max8 covers the pair, halving the
DVE instruction count.  Hardest-positive minima read the masked halves
directly.  Per-row losses leave the device as a [128, m_chunks] tile
(partition p, row chunk m) in one contiguous DMA; the host transposes and
does the masked mean.
"""

import functools

import numpy as np
import ml_dtypes

import concourse.bacc as bacc
import concourse.tile as tile
from concourse import mybir
from concourse.bass_utils import run_bass_kernel_spmd

FP8E4 = mybir.dt.float8e4
FP8E5 = mybir.dt.float8e5
F32 = mybir.dt.float32
BF16 = mybir.dt.bfloat16
E4 = ml_dtypes.float8_e4m3
E5 = ml_dtypes.float8_e5m2

N, D, C = 8192, 1024, 128
NCORES = 8
P = 128
JB = 512  # matmul moving free dim = one fp32 PSUM bank
HPJ = 3   # j-blocks that can contain positive (own-class) columns
ALPHA = 0.1
EPS = 1e-7
BIG = 8192.0
NNORM = 3   # e4m3 norm-expansion slots stolen from the contraction
NSCALE = 8.0  # A-side constant: each slot contributes -8 * e4m3(r/8)
              # (this e4m3 flavor saturates at 240, so r0/8 <= ~165 fits)
MODE = "fp8"


def build_program(rows, n, d, c, jb, mode=MODE, psum_bufs=4, b_bufs=5):
    """Emit the per-core Bass/Tile program (identical on all cores)."""
    kh = d // P
    m_chunks = rows // P
    nj = n // jb
    hpj = min(nj, HPJ)
    assert rows % P == 0 and d % P == 0 and n % jb == 0 and c <= P
    assert kh % 2 == 0 and nj % 2 == 0

    nc = bacc.Bacc("TRN2", target_bir_lowering=False)
    A4 = nc.dram_tensor("A4", [P, kh, rows], FP8E4, kind="ExternalInput")
    B4 = nc.dram_tensor("B4", [P, kh, n], FP8E4, kind="ExternalInput")
    A5 = nc.dram_tensor("A5", [P, 2, rows], FP8E5, kind="ExternalInput")
    B5 = nc.dram_tensor("B5", [P, 2, hpj * jb], FP8E5, kind="ExternalInput")
    loss = nc.dram_tensor("loss", [P, m_chunks], F32, kind="ExternalOutput")

    with tile.TileContext(nc) as tc:
        with (
            tc.tile_pool(name="apool", bufs=1) as apool,
            tc.tile_pool(name="bpool", bufs=b_bufs) as bpool,
            tc.tile_pool(name="psum", bufs=psum_bufs, space="PSUM") as pp,
            tc.tile_pool(name="mpool", bufs=1) as mpool,
            tc.tile_pool(name="fpool", bufs=6) as fpool,
        ):
            # Warm the PE HAM clock gate while the first DMAs land: dummy
            # matmuls on a zeroed tile keep the PE busy through its
            # 4096-cycle activity window so real matmuls run at 2.4 GHz.
            wsrc = apool.tile([1, 16 + jb], BF16, tag="wsrc")
            nc.vector.memset(wsrc[:], 0.0)
            # ~45 short matmuls keep the PE continuously busy (full p-state)
            # until the first operands land, with ~115ns handoff granularity.
            wps = pp.tile([P, 2 * jb], F32, name="ps", tag="ps")
            for _ in range(45):
                nc.tensor.matmul(wps[:16, :P], wsrc[:1, :16],
                                 wsrc[:1, 16:16 + P],
                                 start=True, stop=True)

            kq = kh // 2  # DoubleRow matmuls (k-tile pairs) per data block

            # DMA triggers cost ~0.8us of issuing-engine time each, so the
            # head-critical loads are spread over three otherwise-idle
            # queues: Sync streams B, Scalar streams the stationary A
            # chunks, GpSimd takes the small mask operands.
            def load_bpair(p):
                """Load j-blocks (2p, 2p+1) as one DMA; [[APs], [APs]]."""
                js = slice(2 * p * jb, (2 * p + 2) * jb)
                b4 = bpool.tile([P, kh, 2 * jb], FP8E4, tag="b4", name="b4")
                nc.sync.dma_start(out=b4[:], in_=B4[:, :, js])
                return [[b4[:, 2 * t:2 * t + 2, h * jb:(h + 1) * jb]
                         for t in range(kq)] for h in range(2)]

            def load_b1(j):
                """Load a single j-block as one DMA (head blocks)."""
                js = slice(j * jb, (j + 1) * jb)
                b4 = bpool.tile([P, kh, jb], FP8E4, tag="b4s", name="b4s")
                nc.sync.dma_start(out=b4[:], in_=B4[:, :, js])
                return [b4[:, 2 * t:2 * t + 2, :] for t in range(kq)]

            # Process an UNMASKED pair first: its tiles need no mask
            # operands, so the small A5/B5 transfers get ~20us of slack on
            # the (slow-to-start) GpSimd queue, and the masked pairs run in
            # windows 1-2 -- early enough that their hp minima stay clear
            # of the final-merge window.
            order = [2, 0, 1] + list(range(3, nj // 2))

            # Head loads, one DMA per tensor, interleaved on the Sync queue
            # in consumption order.  Trigger issue (~0.75us apiece on Sync;
            # ~3.5us on Scalar, so everything head-critical stays on Sync)
            # dominates the head, so fewer/bigger DMAs beat split ones.
            bpair = [None] * (nj // 2)
            b0 = load_b1(2 * order[0])
            a4aps = []
            for m in range(m_chunks):
                ms = slice(m * P, (m + 1) * P)
                t = apool.tile([P, kh, P], FP8E4, tag=f"a4m{m}",
                               name=f"a4m{m}")
                nc.sync.dma_start(out=t[:], in_=A4[:, :, ms])
                a4aps.append([t[:, 2 * k:2 * k + 2, :] for k in range(kq)])
                if m == 0:
                    b5all = apool.tile([P, 2, hpj * jb], FP8E5, tag="b5all")
                    nc.gpsimd.dma_start(out=b5all[:], in_=B5[:])
                    b1 = load_b1(2 * order[0] + 1)
                    a5 = apool.tile([P, 2, rows], FP8E5, tag="a5", name="a5")
                    nc.gpsimd.dma_start(out=a5[:], in_=A5[:])
            bpair[order[0]] = [b0, b1]

            def b5ap(j):
                return b5all[:, :, j * jb:(j + 1) * jb]

            # Per-row-chunk partial mining results, merged after the j loop.
            # Max8 runs once per PSUM pair of j-blocks.
            v8 = [mpool.tile([P, nj * 4], F32, tag=f"v8_{m}", name=f"v8_{m}")
                  for m in range(m_chunks)]
            gmin = [mpool.tile([P, hpj], F32, tag=f"gm_{m}", name=f"gmin_{m}")
                    for m in range(m_chunks)]

            stage8 = mpool.tile([P, m_chunks], F32, tag="stage8")
            # hp mining of the masked halves is deferred: the Act engine
            # drains each masked half to SBUF during its (DVE-saturated)
            # window, and the [P, jb] MIN reductions run in later windows
            # where the DVE is half idle.
            pend_min = []

            for w, jp in enumerate(order):
                if bpair[jp] is None:
                    bpair[jp] = load_bpair(jp)
                if w + 1 < len(order) and bpair[order[w + 1]] is None:
                    # Keep the moving stream one pair ahead of the PE.
                    bpair[order[w + 1]] = load_bpair(order[w + 1])
                last = w == len(order) - 1

                for m in range(m_chunks):
                    ps = pp.tile([P, 2 * jb], F32, name="ps", tag="ps")
                    at = a4aps[m]
                    for half in range(2):
                        j = 2 * jp + half
                        b4aps = bpair[jp][half]
                        ph = ps[:, half * jb:(half + 1) * jb]
                        for t in range(kq):
                            nc.tensor.matmul(
                                ph, at[t], b4aps[t],
                                start=(t == 0),
                                stop=(t == kq - 1 and j >= hpj),
                                perf_mode=mybir.MatmulPerfMode.DoubleRow)
                        if j < hpj:
                            # Masked block: -BIG one-hot rides a 5th matmul.
                            nc.tensor.matmul(
                                ph, a5[:, :, m * P:(m + 1) * P], b5ap(j),
                                start=False, stop=True,
                                perf_mode=mybir.MatmulPerfMode.DoubleRow)
                            sc = mpool.tile([P, jb], F32, tag=f"sc{m}_{j}",
                                            name=f"sc{m}_{j}")
                            nc.scalar.copy(sc[:], ph)
                            pend_min.append((sc, m, j))

                    nc.vector.max(v8[m][:, jp * 8:(jp + 1) * 8], ps[:])

                    if 3 <= w < 6 and pend_min:
                        sc, mm, mj = pend_min.pop(0)
                        nc.vector.tensor_reduce(gmin[mm][:, mj:mj + 1], sc[:],
                                                axis=mybir.AxisListType.X,
                                                op=mybir.AluOpType.min)

                    if last:
                        # Final merge for this row chunk, interleaved so it
                        # overlaps the remaining row chunks' matmuls.
                        vf = fpool.tile([P, 8], F32, tag="vf", name="vf")
                        nc.vector.max(vf[:], v8[m][:])
                        gm = fpool.tile([P, 1], F32, tag="gm", name="gm")
                        nc.vector.tensor_reduce(gm[:], gmin[m][:],
                                                axis=mybir.AxisListType.X,
                                                op=mybir.AluOpType.min)
                        # loss_pre = (v2 + (ALPHA - BIG)) - gmin
                        nc.vector.scalar_tensor_tensor(
                            out=stage8[:, m:m + 1], in0=vf[:, 1:2],
                            scalar=float(ALPHA - BIG), in1=gm[:],
                            op0=mybir.AluOpType.add,
                            op1=mybir.AluOpType.subtract)
                        if m == m_chunks // 2 - 1:
                            # First output half leaves while the rest of the
                            # row chunks are still merging.
                            nc.scalar.dma_start(
                                out=loss[:, :m_chunks // 2],
                                in_=stage8[:, :m_chunks // 2])

            assert not pend_min, "deferred hp minima must drain before merge"
            nc.sync.dma_start(out=loss[:, m_chunks // 2:],
                              in_=stage8[:, m_chunks // 2:])

    nc.compile()
    return nc


def _split_e4(x, terms, scale):
    """Greedy expansion: x ~ scale * sum of `terms` e4m3 rows (f64 in/out)."""
    out = []
    r = x.astype(np.float64).copy()
    for _ in range(terms):
        s = (r / scale).astype(E4)
        out.append(s)
        r -= scale * s.astype(np.float64)
    return out


def make_inputs(H, labels, n, d, c, ncores, mode=MODE):
    """Host-side packing of the augmented GEMM operands.

    Rows are sorted by label and sharded contiguously.  Each core's B
    columns are rotated so every column whose label appears among that
    core's rows sits in the leading block (always < HPJ * JB columns), which
    lets the device mine the hardest positive from the first HPJ j-blocks
    only and skip the mask matmul everywhere else.  The final masked mean
    is permutation invariant, so neither the sort nor the rotations need
    undoing.

    The last NNORM contraction slots of A4/B4 are repurposed: A-side holds
    the constant -NSCALE, B-side the greedy e4m3 expansion of ||h_j||^2
    (computed over the SURVIVING d - NNORM dims' quantized values plus the
    full-precision tail, see below), so p_ij needs no extra matmul.
    """
    H = np.ascontiguousarray(np.asarray(H, dtype=np.float32))
    labels = np.asarray(labels).astype(np.int64).ravel()
    kh = d // P
    rows = n // ncores
    nj = n // JB
    hpj = min(nj, HPJ)

    perm = np.argsort(labels, kind="stable")
    H = H[perm]
    labels = labels[perm]
    col_orders = []
    for cix in range(ncores):
        own = np.zeros(n, dtype=bool)
        own[np.isin(labels, labels[cix * rows:(cix + 1) * rows])] = True
        order = np.concatenate([np.nonzero(own)[0], np.nonzero(~own)[0]])
        assert own.sum() <= min(n, hpj * JB), own.sum()
        col_orders.append(order)

    oh = labels[None, :] == np.arange(c, dtype=np.int64)[:, None]  # [c, n]

    Hr = H.astype(E4)
    # Full-data norm (all d dims, at e4m3 precision) -- matches the
    # reference's ||h||^2 term; the dot product just loses the last NNORM
    # dims (zero-mean noise on each distance).
    xn = np.einsum("ij,ij->i", Hr.astype(np.float64), Hr.astype(np.float64))
    xsplit = _split_e4(xn, NNORM, NSCALE)

    B4m = Hr.T.reshape(kh, P, n).transpose(1, 0, 2).copy()  # [P, kh, n] e4m3
    for t in range(NNORM):
        B4m[P - NNORM + t, kh - 1, :] = xsplit[t]
    B5m = np.zeros((P, 2, n), dtype=E5)
    B5m[:c, 0, :] = oh.astype(E5)

    in_maps = []
    for cix in range(ncores):
        sl = slice(cix * rows, (cix + 1) * rows)
        order = col_orders[cix]
        A4m = ((2.0 * Hr.astype(np.float32)[sl].T).astype(E4)
               .reshape(kh, P, rows).transpose(1, 0, 2).copy())
        A4m[P - NNORM:, kh - 1, :] = -NSCALE
        A5m = np.zeros((P, 2, rows), dtype=E5)
        A5m[:c, 0, :] = (-BIG * oh[:, sl]).astype(E5)
        in_maps.append({"A4": A4m, "B4": B4m[:, :, order],
                        "A5": A5m, "B5": B5m[:, :, order[:hpj * JB]]})
    return in_maps


@functools.lru_cache(maxsize=2)
def _get_program(mode=MODE):
    return build_program(N // NCORES, N, D, C, JB, mode=mode)


def _finalize(loss_rows):
    loss_all = np.concatenate(
        [np.asarray(l, dtype=np.float64).T.ravel() for l in loss_rows])
    loss_all = np.maximum(loss_all, 0.0)
    rel = loss_all > EPS
    cnt = int(rel.sum())
    if cnt == 0:
        return np.float32(0.0)
    return np.float32(loss_all[rel].sum() / cnt)


def kernel(H, labels):
    in_maps = make_inputs(H, labels, N, D, C, NCORES)
    res = run_bass_kernel_spmd(_get_program(), in_maps, list(range(NCORES)))
    return _finalize([r["loss"] for r in res.results])


# revision 28
# speedup vs baseline: 1.0100x; 1.0049x over previous
"""Batch-hard triplet loss on 8 Trainium2 NeuronCores.

Math (matches the reference up to fp rounding and a tiny truncation noise):
  d_ij   = ||h_i||^2 + ||h_j||^2 - 2 h_i.h_j, clamped to [EPS, inf)
  hp_i   = max over j (same label, j != i) of d_ij
  hn_i   = 2nd-smallest over j (different label) of d_ij
  loss_i = max(hp_i - hn_i + ALPHA, 0)
  out    = sum(loss_i[loss_i > EPS]) / count(loss_i > EPS)

Device strategy: rows are sharded over 8 cores (1024 each). Each core mines
from the quantity

  p_ij = 2 h_i.h_j - ||h_j||^2 - BIG * [label_i == label_j]

Row-constant terms (||h_i||^2, the EPS clamp) cancel in hp - hn, so they are
never computed.  With t_ij := d_ij - ||h_i||^2 = -p_ij - BIG*eq:
  hp_i = -min_j(p_ij) - BIG        (positives carry -BIG, dominate the min;
                                    Sterbenz: the BIG subtraction is exact)
  hn_i = -max8(p_i)[1]             (negatives are the largest p; the DVE Max8
                                    instruction gives the top-8 descending, so
                                    element 1 is the 2nd-smallest distance,
                                    with tie multiplicity matching top_k)
  loss_i = max( max8[1] - min + (ALPHA - BIG), 0 )   (clamp applied on host)

The key trick: the whole p_ij for an unmasked block comes out of FOUR e4m3
DoubleRow matmuls (K=256 each).  The last 3 of the 1024 contraction slots
carry not data but a 3-term e4m3 expansion of ||h_j||^2 against a constant
-4 on the A side (4*e4m3(x/4) greedy residuals leave |err| <= 1/16), so the
per-column norm rides the GEMM for free.  The 3 sacrificed data dims add
zero-mean noise (std ~3.5) to each distance, far below the mining gaps.

Rows are sorted by label and each core's B columns are rotated so every
own-class (positive) column sits in the first hpj = 3 j-blocks.  Only those
blocks append a 5th e5m2 DoubleRow matmul with the -BIG one-hot mask.

The j loop walks PAIRS of 512-column blocks per row chunk so each PSUM tile
spans two banks ([128, 1024] f32); one DVE Max8 covers the pair, halving the
DVE instruction count.  Hardest-positive minima read the masked halves
directly.  Per-row losses leave the device as a [128, m_chunks] tile
(partition p, row chunk m) in one contiguous DMA; the host transposes and
does the masked mean.
"""

import functools

import numpy as np
import ml_dtypes

import concourse.bacc as bacc
import concourse.tile as tile
from concourse import mybir
from concourse.bass_utils import run_bass_kernel_spmd

FP8E4 = mybir.dt.float8e4
FP8E5 = mybir.dt.float8e5
F32 = mybir.dt.float32
BF16 = mybir.dt.bfloat16
E4 = ml_dtypes.float8_e4m3
E5 = ml_dtypes.float8_e5m2

N, D, C = 8192, 1024, 128
NCORES = 8
P = 128
JB = 512  # matmul moving free dim = one fp32 PSUM bank
HPJ = 3   # j-blocks that can contain positive (own-class) columns
ALPHA = 0.1
EPS = 1e-7
BIG = 8192.0
NNORM = 3   # e4m3 norm-expansion slots stolen from the contraction
NSCALE = 8.0  # A-side constant: each slot contributes -8 * e4m3(r/8)
              # (this e4m3 flavor saturates at 240, so r0/8 <= ~165 fits)
MODE = "fp8"


def build_program(rows, n, d, c, jb, mode=MODE, psum_bufs=4, b_bufs=5):
    """Emit the per-core Bass/Tile program (identical on all cores)."""
    kh = d // P
    m_chunks = rows // P
    nj = n // jb
    hpj = min(nj, HPJ)
    assert rows % P == 0 and d % P == 0 and n % jb == 0 and c <= P
    assert kh % 2 == 0 and nj % 2 == 0

    nc = bacc.Bacc("TRN2", target_bir_lowering=False)
    A4 = nc.dram_tensor("A4", [P, kh, rows], FP8E4, kind="ExternalInput")
    B4 = nc.dram_tensor("B4", [P, kh, n], FP8E4, kind="ExternalInput")
    A5 = nc.dram_tensor("A5", [P, 2, rows], FP8E5, kind="ExternalInput")
    B5 = nc.dram_tensor("B5", [P, 2, hpj * jb], FP8E5, kind="ExternalInput")
    loss = nc.dram_tensor("loss", [P, m_chunks], F32, kind="ExternalOutput")

    with tile.TileContext(nc) as tc:
        with (
            tc.tile_pool(name="apool", bufs=1) as apool,
            tc.tile_pool(name="bpool", bufs=b_bufs) as bpool,
            tc.tile_pool(name="psum", bufs=psum_bufs, space="PSUM") as pp,
            tc.tile_pool(name="mpool", bufs=1) as mpool,
            tc.tile_pool(name="fpool", bufs=6) as fpool,
        ):
            # Warm the PE HAM clock gate while the first DMAs land: dummy
            # matmuls on a zeroed tile keep the PE busy through its
            # 4096-cycle activity window so real matmuls run at 2.4 GHz.
            wsrc = apool.tile([1, 16 + jb], BF16, tag="wsrc")
            nc.vector.memset(wsrc[:], 0.0)
            # ~65 short matmuls keep the PE continuously busy (full p-state)
            # until the first operands land (~14.5us), with ~115ns handoff
            # granularity.
            wps = pp.tile([P, 2 * jb], F32, name="ps", tag="ps")
            for _ in range(65):
                nc.tensor.matmul(wps[:16, :P], wsrc[:1, :16],
                                 wsrc[:1, 16:16 + P],
                                 start=True, stop=True)

            kq = kh // 2  # DoubleRow matmuls (k-tile pairs) per data block

            # DMA triggers cost ~0.8us of issuing-engine time each, so the
            # head-critical loads are spread over three otherwise-idle
            # queues: Sync streams B, Scalar streams the stationary A
            # chunks, GpSimd takes the small mask operands.
            def load_bpair(p):
                """Load j-blocks (2p, 2p+1) as one DMA; [[APs], [APs]]."""
                js = slice(2 * p * jb, (2 * p + 2) * jb)
                b4 = bpool.tile([P, kh, 2 * jb], FP8E4, tag="b4", name="b4")
                nc.sync.dma_start(out=b4[:], in_=B4[:, :, js])
                return [[b4[:, 2 * t:2 * t + 2, h * jb:(h + 1) * jb]
                         for t in range(kq)] for h in range(2)]

            def load_b1(j):
                """Load a single j-block as one DMA (head blocks)."""
                js = slice(j * jb, (j + 1) * jb)
                b4 = bpool.tile([P, kh, jb], FP8E4, tag="b4s", name="b4s")
                nc.sync.dma_start(out=b4[:], in_=B4[:, :, js])
                return [b4[:, 2 * t:2 * t + 2, :] for t in range(kq)]

            # Process an UNMASKED pair first: its tiles need no mask
            # operands, so the small A5/B5 transfers get ~20us of slack on
            # the (slow-to-start) GpSimd queue, and the masked pairs run in
            # windows 1-2 -- early enough that their hp minima stay clear
            # of the final-merge window.
            order = [2, 0, 1] + list(range(3, nj // 2))

            # Head loads, one DMA per tensor, interleaved on the Sync queue
            # in consumption order.  Trigger issue (~0.75us apiece on Sync;
            # ~3.5us on Scalar, so everything head-critical stays on Sync)
            # dominates the head, so fewer/bigger DMAs beat split ones.
            bpair = [None] * (nj // 2)
            b0 = load_b1(2 * order[0])
            a4aps = []
            for m in range(m_chunks):
                ms = slice(m * P, (m + 1) * P)
                t = apool.tile([P, kh, P], FP8E4, tag=f"a4m{m}",
                               name=f"a4m{m}")
                nc.sync.dma_start(out=t[:], in_=A4[:, :, ms])
                a4aps.append([t[:, 2 * k:2 * k + 2, :] for k in range(kq)])
                if m == 0:
                    b5all = apool.tile([P, 2, hpj * jb], FP8E5, tag="b5all")
                    nc.gpsimd.dma_start(out=b5all[:], in_=B5[:])
                    b1 = load_b1(2 * order[0] + 1)
                    a5 = apool.tile([P, 2, rows], FP8E5, tag="a5", name="a5")
                    nc.gpsimd.dma_start(out=a5[:], in_=A5[:])
            bpair[order[0]] = [b0, b1]

            def b5ap(j):
                return b5all[:, :, j * jb:(j + 1) * jb]

            # Per-row-chunk partial mining results, merged after the j loop.
            # Max8 runs once per PSUM pair of j-blocks.
            v8 = [mpool.tile([P, nj * 4], F32, tag=f"v8_{m}", name=f"v8_{m}")
                  for m in range(m_chunks)]
            gmin = [mpool.tile([P, hpj], F32, tag=f"gm_{m}", name=f"gmin_{m}")
                    for m in range(m_chunks)]

            stage8 = mpool.tile([P, m_chunks], F32, tag="stage8")
            # hp mining of the masked halves is deferred: the Act engine
            # drains each masked half to SBUF during its (DVE-saturated)
            # window, and the [P, jb] MIN reductions run in later windows
            # where the DVE is half idle.
            pend_min = []

            for w, jp in enumerate(order):
                if bpair[jp] is None:
                    bpair[jp] = load_bpair(jp)
                if w + 1 < len(order) and bpair[order[w + 1]] is None:
                    # Keep the moving stream one pair ahead of the PE.
                    bpair[order[w + 1]] = load_bpair(order[w + 1])
                last = w == len(order) - 1

                for m in range(m_chunks):
                    ps = pp.tile([P, 2 * jb], F32, name="ps", tag="ps")
                    at = a4aps[m]
                    for half in range(2):
                        j = 2 * jp + half
                        b4aps = bpair[jp][half]
                        ph = ps[:, half * jb:(half + 1) * jb]
                        for t in range(kq):
                            nc.tensor.matmul(
                                ph, at[t], b4aps[t],
                                start=(t == 0),
                                stop=(t == kq - 1 and j >= hpj),
                                perf_mode=mybir.MatmulPerfMode.DoubleRow)
                        if j < hpj:
                            # Masked block: -BIG one-hot rides a 5th matmul.
                            nc.tensor.matmul(
                                ph, a5[:, :, m * P:(m + 1) * P], b5ap(j),
                                start=False, stop=True,
                                perf_mode=mybir.MatmulPerfMode.DoubleRow)
                            sc = mpool.tile([P, jb], F32, tag=f"sc{m}_{j}",
                                            name=f"sc{m}_{j}")
                            nc.scalar.copy(sc[:], ph)
                            pend_min.append((sc, m, j))

                    nc.vector.max(v8[m][:, jp * 8:(jp + 1) * 8], ps[:])

                    if 3 <= w < 6 and pend_min:
                        sc, mm, mj = pend_min.pop(0)
                        nc.vector.tensor_reduce(gmin[mm][:, mj:mj + 1], sc[:],
                                                axis=mybir.AxisListType.X,
                                                op=mybir.AluOpType.min)

                    if last:
                        # Final merge for this row chunk, interleaved so it
                        # overlaps the remaining row chunks' matmuls.
                        vf = fpool.tile([P, 8], F32, tag="vf", name="vf")
                        nc.vector.max(vf[:], v8[m][:])
                        gm = fpool.tile([P, 1], F32, tag="gm", name="gm")
                        nc.vector.tensor_reduce(gm[:], gmin[m][:],
                                                axis=mybir.AxisListType.X,
                                                op=mybir.AluOpType.min)
                        # loss_pre = (v2 + (ALPHA - BIG)) - gmin
                        nc.vector.scalar_tensor_tensor(
                            out=stage8[:, m:m + 1], in0=vf[:, 1:2],
                            scalar=float(ALPHA - BIG), in1=gm[:],
                            op0=mybir.AluOpType.add,
                            op1=mybir.AluOpType.subtract)
                        if m == m_chunks // 2 - 1:
                            # First output half leaves while the rest of the
                            # row chunks are still merging.
                            nc.scalar.dma_start(
                                out=loss[:, :m_chunks // 2],
                                in_=stage8[:, :m_chunks // 2])

            assert not pend_min, "deferred hp minima must drain before merge"
            nc.sync.dma_start(out=loss[:, m_chunks // 2:],
                              in_=stage8[:, m_chunks // 2:])

    nc.compile()
    return nc


def _split_e4(x, terms, scale):
    """Greedy expansion: x ~ scale * sum of `terms` e4m3 rows (f64 in/out)."""
    out = []
    r = x.astype(np.float64).copy()
    for _ in range(terms):
        s = (r / scale).astype(E4)
        out.append(s)
        r -= scale * s.astype(np.float64)
    return out


def make_inputs(H, labels, n, d, c, ncores, mode=MODE):
    """Host-side packing of the augmented GEMM operands.

    Rows are sorted by label and sharded contiguously.  Each core's B
    columns are rotated so every column whose label appears among that
    core's rows sits in the leading block (always < HPJ * JB columns), which
    lets the device mine the hardest positive from the first HPJ j-blocks
    only and skip the mask matmul everywhere else.  The final masked mean
    is permutation invariant, so neither the sort nor the rotations need
    undoing.

    The last NNORM contraction slots of A4/B4 are repurposed: A-side holds
    the constant -NSCALE, B-side the greedy e4m3 expansion of ||h_j||^2
    (computed over the SURVIVING d - NNORM dims' quantized values plus the
    full-precision tail, see below), so p_ij needs no extra matmul.
    """
    H = np.ascontiguousarray(np.asarray(H, dtype=np.float32))
    labels = np.asarray(labels).astype(np.int64).ravel()
    kh = d // P
    rows = n // ncores
    nj = n // JB
    hpj = min(nj, HPJ)

    perm = np.argsort(labels, kind="stable")
    H = H[perm]
    labels = labels[perm]
    col_orders = []
    for cix in range(ncores):
        own = np.zeros(n, dtype=bool)
        own[np.isin(labels, labels[cix * rows:(cix + 1) * rows])] = True
        order = np.concatenate([np.nonzero(own)[0], np.nonzero(~own)[0]])
        assert own.sum() <= min(n, hpj * JB), own.sum()
        col_orders.append(order)

    oh = labels[None, :] == np.arange(c, dtype=np.int64)[:, None]  # [c, n]

    Hr = H.astype(E4)
    # Full-data norm (all d dims, at e4m3 precision) -- matches the
    # reference's ||h||^2 term; the dot product just loses the last NNORM
    # dims (zero-mean noise on each distance).
    xn = np.einsum("ij,ij->i", Hr.astype(np.float64), Hr.astype(np.float64))
    xsplit = _split_e4(xn, NNORM, NSCALE)

    B4m = Hr.T.reshape(kh, P, n).transpose(1, 0, 2).copy()  # [P, kh, n] e4m3
    for t in range(NNORM):
        B4m[P - NNORM + t, kh - 1, :] = xsplit[t]
    B5m = np.zeros((P, 2, n), dtype=E5)
    B5m[:c, 0, :] = oh.astype(E5)

    in_maps = []
    for cix in range(ncores):
        sl = slice(cix * rows, (cix + 1) * rows)
        order = col_orders[cix]
        A4m = ((2.0 * Hr.astype(np.float32)[sl].T).astype(E4)
               .reshape(kh, P, rows).transpose(1, 0, 2).copy())
        A4m[P - NNORM:, kh - 1, :] = -NSCALE
        A5m = np.zeros((P, 2, rows), dtype=E5)
        A5m[:c, 0, :] = (-BIG * oh[:, sl]).astype(E5)
        in_maps.append({"A4": A4m, "B4": B4m[:, :, order],
                        "A5": A5m, "B5": B5m[:, :, order[:hpj * JB]]})
    return in_maps


@functools.lru_cache(maxsize=2)
def _get_program(mode=MODE):
    return build_program(N // NCORES, N, D, C, JB, mode=mode)


def _finalize(loss_rows):
    loss_all = np.concatenate(
        [np.asarray(l, dtype=np.float64).T.ravel() for l in loss_rows])
    loss_all = np.maximum(loss_all, 0.0)
    rel = loss_all > EPS
    cnt = int(rel.sum())
    if cnt == 0:
        return np.float32(0.0)
    return np.float32(loss_all[rel].sum() / cnt)


def kernel(H, labels):
    in_maps = make_inputs(H, labels, N, D, C, NCORES)
    res = run_bass_kernel_spmd(_get_program(), in_maps, list(range(NCORES)))
    return _finalize([r["loss"] for r in res.results])


# revision 29
# speedup vs baseline: 1.0219x; 1.0118x over previous
"""Batch-hard triplet loss on 8 Trainium2 NeuronCores.

Math (matches the reference up to fp rounding and a tiny truncation noise):
  d_ij   = ||h_i||^2 + ||h_j||^2 - 2 h_i.h_j, clamped to [EPS, inf)
  hp_i   = max over j (same label, j != i) of d_ij
  hn_i   = 2nd-smallest over j (different label) of d_ij
  loss_i = max(hp_i - hn_i + ALPHA, 0)
  out    = sum(loss_i[loss_i > EPS]) / count(loss_i > EPS)

Device strategy: rows are sharded over 8 cores (1024 each). Each core mines
from the quantity

  p_ij = 2 h_i.h_j - ||h_j||^2 - BIG * [label_i == label_j]

Row-constant terms (||h_i||^2, the EPS clamp) cancel in hp - hn, so they are
never computed.  With t_ij := d_ij - ||h_i||^2 = -p_ij - BIG*eq:
  hp_i = -min_j(p_ij) - BIG        (positives carry -BIG, dominate the min;
                                    Sterbenz: the BIG subtraction is exact)
  hn_i = -max8(p_i)[1]             (negatives are the largest p; the DVE Max8
                                    instruction gives the top-8 descending, so
                                    element 1 is the 2nd-smallest distance,
                                    with tie multiplicity matching top_k)
  loss_i = max( max8[1] - min + (ALPHA - BIG), 0 )   (clamp applied on host)

The key trick: the whole p_ij for an unmasked block comes out of FOUR e4m3
DoubleRow matmuls (K=256 each).  The last 3 of the 1024 contraction slots
carry not data but a 3-term e4m3 expansion of ||h_j||^2 against a constant
-4 on the A side (4*e4m3(x/4) greedy residuals leave |err| <= 1/16), so the
per-column norm rides the GEMM for free.  The 3 sacrificed data dims add
zero-mean noise (std ~3.5) to each distance, far below the mining gaps.

Rows are sorted by label and each core's B columns are rotated so every
own-class (positive) column sits in the first hpj = 3 j-blocks.  Only those
blocks append a 5th e5m2 DoubleRow matmul with the -BIG one-hot mask.

The j loop walks PAIRS of 512-column blocks per row chunk so each PSUM tile
spans two banks ([128, 1024] f32); one DVE Max8 covers the pair, halving the
DVE instruction count.  Hardest-positive minima read the masked halves
directly.  Per-row losses leave the device as a [128, m_chunks] tile
(partition p, row chunk m) in one contiguous DMA; the host transposes and
does the masked mean.
"""

import functools

import numpy as np
import ml_dtypes

import concourse.bacc as bacc
import concourse.tile as tile
from concourse import mybir
from concourse.bass_utils import run_bass_kernel_spmd

FP8E4 = mybir.dt.float8e4
FP8E5 = mybir.dt.float8e5
F32 = mybir.dt.float32
BF16 = mybir.dt.bfloat16
E4 = ml_dtypes.float8_e4m3
E5 = ml_dtypes.float8_e5m2

N, D, C = 8192, 1024, 128
NCORES = 8
P = 128
JB = 512  # matmul moving free dim = one fp32 PSUM bank
HPJ = 3   # j-blocks that can contain positive (own-class) columns
ALPHA = 0.1
EPS = 1e-7
BIG = 8192.0
NNORM = 3   # e4m3 norm-expansion slots stolen from the contraction
NSCALE = 8.0  # A-side constant: each slot contributes -8 * e4m3(r/8)
              # (this e4m3 flavor saturates at 240, so r0/8 <= ~165 fits)
MODE = "fp8"


def build_program(rows, n, d, c, jb, mode=MODE, psum_bufs=4, b_bufs=5):
    """Emit the per-core Bass/Tile program (identical on all cores)."""
    kh = d // P
    m_chunks = rows // P
    nj = n // jb
    hpj = min(nj, HPJ)
    assert rows % P == 0 and d % P == 0 and n % jb == 0 and c <= P
    assert kh % 2 == 0 and nj % 2 == 0

    nc = bacc.Bacc("TRN2", target_bir_lowering=False)
    A4 = nc.dram_tensor("A4", [P, kh, rows], FP8E4, kind="ExternalInput")
    B4 = nc.dram_tensor("B4", [P, kh, n], FP8E4, kind="ExternalInput")
    A5 = nc.dram_tensor("A5", [P, 2, rows], FP8E5, kind="ExternalInput")
    B5 = nc.dram_tensor("B5", [P, 2, hpj * jb], FP8E5, kind="ExternalInput")
    loss = nc.dram_tensor("loss", [P, m_chunks], F32, kind="ExternalOutput")

    with tile.TileContext(nc) as tc:
        with (
            tc.tile_pool(name="apool", bufs=1) as apool,
            tc.tile_pool(name="bpool", bufs=b_bufs) as bpool,
            tc.tile_pool(name="psum", bufs=psum_bufs, space="PSUM") as pp,
            tc.tile_pool(name="mpool", bufs=1) as mpool,
            tc.tile_pool(name="fpool", bufs=6) as fpool,
        ):
            # Warm the PE HAM clock gate while the first DMAs land: dummy
            # matmuls on a zeroed tile keep the PE busy through its
            # 4096-cycle activity window so real matmuls run at 2.4 GHz.
            wsrc = apool.tile([P, 2 * P], BF16, tag="wsrc")
            nc.vector.memset(wsrc[:], 0.0)
            # ~65 short FULL-ARRAY matmuls keep all 128x128 PE cells busy
            # (so the HAM power ramp completes during the warmup, not on the
            # first real matmuls) until the first operands land (~14.5us),
            # with ~115ns handoff granularity.
            wps = pp.tile([P, 2 * jb], F32, name="ps", tag="ps")
            for _ in range(65):
                nc.tensor.matmul(wps[:, :P], wsrc[:, :P], wsrc[:, P:],
                                 start=True, stop=True)

            kq = kh // 2  # DoubleRow matmuls (k-tile pairs) per data block

            # DMA triggers cost ~0.8us of issuing-engine time each, so the
            # head-critical loads are spread over three otherwise-idle
            # queues: Sync streams B, Scalar streams the stationary A
            # chunks, GpSimd takes the small mask operands.
            def load_bpair(p):
                """Load j-blocks (2p, 2p+1) as one DMA; [[APs], [APs]]."""
                js = slice(2 * p * jb, (2 * p + 2) * jb)
                b4 = bpool.tile([P, kh, 2 * jb], FP8E4, tag="b4", name="b4")
                nc.sync.dma_start(out=b4[:], in_=B4[:, :, js])
                return [[b4[:, 2 * t:2 * t + 2, h * jb:(h + 1) * jb]
                         for t in range(kq)] for h in range(2)]

            def load_b1(j):
                """Load a single j-block as one DMA (head blocks)."""
                js = slice(j * jb, (j + 1) * jb)
                b4 = bpool.tile([P, kh, jb], FP8E4, tag="b4s", name="b4s")
                nc.sync.dma_start(out=b4[:], in_=B4[:, :, js])
                return [b4[:, 2 * t:2 * t + 2, :] for t in range(kq)]

            # Process an UNMASKED pair first: its tiles need no mask
            # operands, so the small A5/B5 transfers get ~20us of slack on
            # the (slow-to-start) GpSimd queue, and the masked pairs run in
            # windows 1-2 -- early enough that their hp minima stay clear
            # of the final-merge window.
            order = [2, 0, 1] + list(range(3, nj // 2))

            # Head loads, one DMA per tensor, interleaved on the Sync queue
            # in consumption order.  Trigger issue (~0.75us apiece on Sync;
            # ~3.5us on Scalar, so everything head-critical stays on Sync)
            # dominates the head, so fewer/bigger DMAs beat split ones.
            bpair = [None] * (nj // 2)
            b0 = load_b1(2 * order[0])
            a4aps = []
            for m in range(m_chunks):
                ms = slice(m * P, (m + 1) * P)
                t = apool.tile([P, kh, P], FP8E4, tag=f"a4m{m}",
                               name=f"a4m{m}")
                nc.sync.dma_start(out=t[:], in_=A4[:, :, ms])
                a4aps.append([t[:, 2 * k:2 * k + 2, :] for k in range(kq)])
                if m == 0:
                    b5all = apool.tile([P, 2, hpj * jb], FP8E5, tag="b5all")
                    nc.gpsimd.dma_start(out=b5all[:], in_=B5[:])
                    b1 = load_b1(2 * order[0] + 1)
                    a5 = apool.tile([P, 2, rows], FP8E5, tag="a5", name="a5")
                    nc.gpsimd.dma_start(out=a5[:], in_=A5[:])
            bpair[order[0]] = [b0, b1]

            def b5ap(j):
                return b5all[:, :, j * jb:(j + 1) * jb]

            # Per-row-chunk partial mining results, merged after the j loop.
            # Max8 runs once per PSUM pair of j-blocks.
            v8 = [mpool.tile([P, nj * 4], F32, tag=f"v8_{m}", name=f"v8_{m}")
                  for m in range(m_chunks)]
            gmin = [mpool.tile([P, hpj], F32, tag=f"gm_{m}", name=f"gmin_{m}")
                    for m in range(m_chunks)]

            stage8 = mpool.tile([P, m_chunks], F32, tag="stage8")
            # hp mining of the masked halves is deferred: the Act engine
            # drains each masked half to SBUF during its (DVE-saturated)
            # window, and the [P, jb] MIN reductions run in later windows
            # where the DVE is half idle.
            pend_min = []

            for w, jp in enumerate(order):
                if bpair[jp] is None:
                    bpair[jp] = load_bpair(jp)
                if w + 1 < len(order) and bpair[order[w + 1]] is None:
                    # Keep the moving stream one pair ahead of the PE.
                    bpair[order[w + 1]] = load_bpair(order[w + 1])
                last = w == len(order) - 1

                for m in range(m_chunks):
                    ps = pp.tile([P, 2 * jb], F32, name="ps", tag="ps")
                    at = a4aps[m]
                    for half in range(2):
                        j = 2 * jp + half
                        b4aps = bpair[jp][half]
                        ph = ps[:, half * jb:(half + 1) * jb]
                        for t in range(kq):
                            nc.tensor.matmul(
                                ph, at[t], b4aps[t],
                                start=(t == 0),
                                stop=(t == kq - 1 and j >= hpj),
                                perf_mode=mybir.MatmulPerfMode.DoubleRow)
                        if j < hpj:
                            # Masked block: -BIG one-hot rides a 5th matmul.
                            nc.tensor.matmul(
                                ph, a5[:, :, m * P:(m + 1) * P], b5ap(j),
                                start=False, stop=True,
                                perf_mode=mybir.MatmulPerfMode.DoubleRow)
                            sc = mpool.tile([P, jb], F32, tag=f"sc{m}_{j}",
                                            name=f"sc{m}_{j}")
                            nc.scalar.copy(sc[:], ph)
                            pend_min.append((sc, m, j))

                    nc.vector.max(v8[m][:, jp * 8:(jp + 1) * 8], ps[:])

                    if 3 <= w < 6 and pend_min:
                        sc, mm, mj = pend_min.pop(0)
                        nc.vector.tensor_reduce(gmin[mm][:, mj:mj + 1], sc[:],
                                                axis=mybir.AxisListType.X,
                                                op=mybir.AluOpType.min)

                    if last:
                        # Final merge for this row chunk, interleaved so it
                        # overlaps the remaining row chunks' matmuls.
                        vf = fpool.tile([P, 8], F32, tag="vf", name="vf")
                        nc.vector.max(vf[:], v8[m][:])
                        gm = fpool.tile([P, 1], F32, tag="gm", name="gm")
                        nc.vector.tensor_reduce(gm[:], gmin[m][:],
                                                axis=mybir.AxisListType.X,
                                                op=mybir.AluOpType.min)
                        # loss_pre = (v2 + (ALPHA - BIG)) - gmin
                        nc.vector.scalar_tensor_tensor(
                            out=stage8[:, m:m + 1], in0=vf[:, 1:2],
                            scalar=float(ALPHA - BIG), in1=gm[:],
                            op0=mybir.AluOpType.add,
                            op1=mybir.AluOpType.subtract)
                        if m == m_chunks // 2 - 1:
                            # First output half leaves while the rest of the
                            # row chunks are still merging.
                            nc.scalar.dma_start(
                                out=loss[:, :m_chunks // 2],
                                in_=stage8[:, :m_chunks // 2])

            assert not pend_min, "deferred hp minima must drain before merge"
            nc.sync.dma_start(out=loss[:, m_chunks // 2:],
                              in_=stage8[:, m_chunks // 2:])

    nc.compile()
    return nc


def _split_e4(x, terms, scale):
    """Greedy expansion: x ~ scale * sum of `terms` e4m3 rows (f64 in/out)."""
    out = []
    r = x.astype(np.float64).copy()
    for _ in range(terms):
        s = (r / scale).astype(E4)
        out.append(s)
        r -= scale * s.astype(np.float64)
    return out


def make_inputs(H, labels, n, d, c, ncores, mode=MODE):
    """Host-side packing of the augmented GEMM operands.

    Rows are sorted by label and sharded contiguously.  Each core's B
    columns are rotated so every column whose label appears among that
    core's rows sits in the leading block (always < HPJ * JB columns), which
    lets the device mine the hardest positive from the first HPJ j-blocks
    only and skip the mask matmul everywhere else.  The final masked mean
    is permutation invariant, so neither the sort nor the rotations need
    undoing.

    The last NNORM contraction slots of A4/B4 are repurposed: A-side holds
    the constant -NSCALE, B-side the greedy e4m3 expansion of ||h_j||^2
    (computed over the SURVIVING d - NNORM dims' quantized values plus the
    full-precision tail, see below), so p_ij needs no extra matmul.
    """
    H = np.ascontiguousarray(np.asarray(H, dtype=np.float32))
    labels = np.asarray(labels).astype(np.int64).ravel()
    kh = d // P
    rows = n // ncores
    nj = n // JB
    hpj = min(nj, HPJ)

    perm = np.argsort(labels, kind="stable")
    H = H[perm]
    labels = labels[perm]
    col_orders = []
    for cix in range(ncores):
        own = np.zeros(n, dtype=bool)
        own[np.isin(labels, labels[cix * rows:(cix + 1) * rows])] = True
        order = np.concatenate([np.nonzero(own)[0], np.nonzero(~own)[0]])
        assert own.sum() <= min(n, hpj * JB), own.sum()
        col_orders.append(order)

    oh = labels[None, :] == np.arange(c, dtype=np.int64)[:, None]  # [c, n]

    Hr = H.astype(E4)
    # Full-data norm (all d dims, at e4m3 precision) -- matches the
    # reference's ||h||^2 term; the dot product just loses the last NNORM
    # dims (zero-mean noise on each distance).
    xn = np.einsum("ij,ij->i", Hr.astype(np.float64), Hr.astype(np.float64))
    xsplit = _split_e4(xn, NNORM, NSCALE)

    B4m = Hr.T.reshape(kh, P, n).transpose(1, 0, 2).copy()  # [P, kh, n] e4m3
    for t in range(NNORM):
        B4m[P - NNORM + t, kh - 1, :] = xsplit[t]
    B5m = np.zeros((P, 2, n), dtype=E5)
    B5m[:c, 0, :] = oh.astype(E5)

    in_maps = []
    for cix in range(ncores):
        sl = slice(cix * rows, (cix + 1) * rows)
        order = col_orders[cix]
        A4m = ((2.0 * Hr.astype(np.float32)[sl].T).astype(E4)
               .reshape(kh, P, rows).transpose(1, 0, 2).copy())
        A4m[P - NNORM:, kh - 1, :] = -NSCALE
        A5m = np.zeros((P, 2, rows), dtype=E5)
        A5m[:c, 0, :] = (-BIG * oh[:, sl]).astype(E5)
        in_maps.append({"A4": A4m, "B4": B4m[:, :, order],
                        "A5": A5m, "B5": B5m[:, :, order[:hpj * JB]]})
    return in_maps


@functools.lru_cache(maxsize=2)
def _get_program(mode=MODE):
    return build_program(N // NCORES, N, D, C, JB, mode=mode)


def _finalize(loss_rows):
    loss_all = np.concatenate(
        [np.asarray(l, dtype=np.float64).T.ravel() for l in loss_rows])
    loss_all = np.maximum(loss_all, 0.0)
    rel = loss_all > EPS
    cnt = int(rel.sum())
    if cnt == 0:
        return np.float32(0.0)
    return np.float32(loss_all[rel].sum() / cnt)


def kernel(H, labels):
    in_maps = make_inputs(H, labels, N, D, C, NCORES)
    res = run_bass_kernel_spmd(_get_program(), in_maps, list(range(NCORES)))
    return _finalize([r["loss"] for r in res.results])
